# revision 1
# baseline (speedup 1.0000x reference)
"""BiDirectionalCrossAttention Trainium2 kernel (8-core data parallel).

Math (per sample m, matching the reference):
  q1 = x @ Wq1.T + bq1   (x = protein)     k1,v1 from y (ligand)
  q2 = y @ Wq2.T + bq2                     k2,v2 from x
  S[h,e]   = q[h,:] . k[e,:] / sqrt(64)    (heads mix: 8x8 scores per sample)
  A        = softmax_e(S)
  out[h,:] = sum_e A[h,e] v[e,:]
  protein_out = out1 @ Wo1.T + bo1 ; ligand_out = out2 @ Wo2.T + bo2

Mapping:
  - batch is sharded 8 ways (4096 samples/core); weights replicated.
  - inputs are passed transposed (xT [512, 4096]) so the 128-row K-chunks of
    x.T serve directly as matmul stationary operands; biases ride as a 513th
    row combined with a ones-row K=1 matmul.
  - projections + output projections run on the TensorEngine in float32r
    (full rate at N=512, ~1e-4 relative error vs fp32).
  - per-sample attention runs with samples on partitions: broadcast-AP
    (0-stride) products in bf16 on DVE (2x mode), segmented reductions as
    bf16 tree-adds (2x) + a final fp32 tensor_reduce, exp on ScalarE, and
    the A@V product split 6:2 between GPSIMD and DVE to balance engines.
  - softmax normalization is deferred until after the A@V product so the
    GPSIMD launch sits on a short dependency chain.
  - the per-(tile, direction) work is software-pipelined 2-3 stages deep
    across the whole batch (stage1: proj+scores+exp+AV-product; stage2a:
    denominator+AV-reduce+normalize; stage2b: PE-transpose+staging), which
    keeps DVE ~90% busy and avoids FIFO head-of-line blocking on the
    GPSIMD latency.
  - final outputs are produced transposed ([512, 4096] per core, bias added
    via per-partition ACT bias during PSUM evacuation) and un-transposed on
    the host.

Cost-model timeline of the compiled module: ~700 us per core.
"""

import os

import numpy as np

import concourse.bacc as bacc
import concourse.mybir as mybir
import concourse.tile as tile
from concourse import bass_utils

B, NF = 32768, 512
H, DH = 8, 64
NCORES = 8
BC = B // NCORES          # samples per core
MT = 128                  # attention tile (samples)
ST = 512                  # projection super-tile (samples)
N_ST = BC // ST
N_MT = ST // MT
SCALE = 8.0               # sqrt(DH)

f32 = mybir.dt.float32
f32r = mybir.dt.float32r
bf16 = mybir.dt.bfloat16

# score products in bf16 (2x DVE mode); flip off for full fp32 accuracy
PROD_BF16 = bool(int(os.environ.get("BIDIR_PROD_BF16", "1")))
# run the A@V product on GPSIMD to unload the Vector engine
AV_ON_GPSIMD = bool(int(os.environ.get("BIDIR_AV_GPSIMD", "1")))

WNAMES = ["q1", "k1", "v1", "q2", "k2", "v2"]

_CACHE: dict = {}


def _emit(nc, tc, dr):
    from contextlib import ExitStack

    X = mybir.AxisListType.X
    ADD = mybir.AluOpType.add
    EXP = mybir.ActivationFunctionType.Exp
    IDENT_FN = mybir.ActivationFunctionType.Identity

    with ExitStack() as ctx:
        wpool = ctx.enter_context(tc.tile_pool(name="weights", bufs=1))
        xpool = ctx.enter_context(tc.tile_pool(name="xstage", bufs=1))
        qkv_pool = ctx.enter_context(tc.tile_pool(name="qkv", bufs=int(os.environ.get("BIDIR_QKV_BUFS", "2"))))
        big_pool = ctx.enter_context(tc.tile_pool(name="bigp", bufs=int(os.environ.get("BIDIR_BIG_BUFS", "4"))))
        prod_pool = ctx.enter_context(tc.tile_pool(name="prod", bufs=int(os.environ.get("BIDIR_TR_BUFS", "2"))))
        small_pool = ctx.enter_context(tc.tile_pool(name="small", bufs=int(os.environ.get("BIDIR_SMALL_BUFS", "5"))))
        ao_pool = ctx.enter_context(tc.tile_pool(name="aoT", bufs=int(os.environ.get("BIDIR_AO_BUFS", "1"))))
        out_pool = ctx.enter_context(tc.tile_pool(name="outb", bufs=int(os.environ.get("BIDIR_OUT_BUFS", "5"))))
        pp = ctx.enter_context(tc.tile_pool(name="pproj", bufs=2, space="PSUM"))
        pt = ctx.enter_context(tc.tile_pool(name="ptrans", bufs=1, space="PSUM"))
        po = ctx.enter_context(tc.tile_pool(name="pout", bufs=1, space="PSUM"))

        # ---- static weights ----
        W = {}
        for n in WNAMES:
            chunks = []
            for c in range(4):
                t = wpool.tile([128, NF], f32r, tag=f"w_{n}_{c}", name=f"w_{n}_{c}")
                nc.sync.dma_start(t[:], dr[f"w_{n}"].ap()[128 * c:128 * (c + 1), :])
                chunks.append(t)
            bt = wpool.tile([1, NF], f32r, tag=f"w_{n}_b", name=f"w_{n}_b")
            nc.sync.dma_start(bt[:], dr[f"w_{n}"].ap()[NF:NF + 1, :])
            W[n] = (chunks, bt)
        WO = {}
        for n in ("o1", "o2"):
            WO[n] = []
            for c in range(4):
                t = wpool.tile([128, NF], f32r, tag=f"wo_{n}_{c}", name=f"wo_{n}_{c}")
                nc.sync.dma_start(t[:], dr[f"w{n}T"].ap()[128 * c:128 * (c + 1), :])
                WO[n].append(t)
        bo_sb = {}
        for n in ("o1", "o2"):
            t = wpool.tile([128, 4], f32, tag=f"bo_{n}", name=f"bo_{n}")
            nc.sync.dma_start(t[:], dr[f"b{n}c"].ap())
            bo_sb[n] = t
        ones = wpool.tile([1, MT], f32r, tag="ones", name="ones")
        nc.sync.dma_start(ones[:], dr["ones_row"].ap())
        ident = wpool.tile([128, 128], f32, tag="ident", name="ident")
        nc.sync.dma_start(ident[:], dr["ident"].ap())

        qk_dt = bf16 if PROD_BF16 else f32
        p2_dt = bf16 if PROD_BF16 else f32

        def load_supertile(s):
            ssl = slice(ST * s, ST * (s + 1))
            xs, ys = [], []
            for c in range(4):
                xt = xpool.tile([128, ST], f32r, tag=f"xs{c}", name=f"xs{c}")
                nc.sync.dma_start(xt[:], dr["xT"].ap()[128 * c:128 * (c + 1), ssl])
                xs.append(xt)
                yt = xpool.tile([128, ST], f32r, tag=f"ys{c}", name=f"ys{c}")
                nc.sync.dma_start(yt[:], dr["yT"].ap()[128 * c:128 * (c + 1), ssl])
                ys.append(yt)
            aoT = {1: ao_pool.tile([128, 4, ST], f32r, tag="aoT1", name="aoT1"),
                   2: ao_pool.tile([128, 4, ST], f32r, tag="aoT2", name="aoT2")}
            return {"xs": xs, "ys": ys, "aoT": aoT, "s": s, "done": 0}

        def stage1(sup, t, d, qn, kn, vn):
            """projections -> evac -> scores -> softmax -> A@V product."""
            if True:
                xs, ys, aoT = sup["xs"], sup["ys"], sup["aoT"]
                msl = slice(MT * t, MT * (t + 1))
                ps = {}
                pqb = int(os.environ.get("BIDIR_PQ_BUFS", "0"))
                for role, n in (("q", qn), ("k", kn), ("v", vn)):
                    if pqb:
                        bspec = {"q": 3, "k": 2, "v": 1}[role]
                        ps[n] = pp.tile([128, NF], f32, tag=f"p_{role}",
                                        name=f"p_{role}", bufs=bspec)
                    else:
                        ps[n] = pp.tile([128, NF], f32, tag=f"p_{role}", name=f"p_{role}")
                if d == 1:
                    srcs = {qn: xs, kn: ys, vn: ys}
                else:
                    srcs = {qn: ys, kn: xs, vn: xs}
                sb = {}
                # per projection: 4 K-chunks + bias row, then evacuate right
                # away so the scores chain starts as early as possible
                for n in (qn, kn, vn):
                    for c in range(4):
                        nc.tensor.matmul(ps[n][:], srcs[n][c][:, msl],
                                         W[n][0][c][:],
                                         start=(c == 0), stop=False)
                    nc.tensor.matmul(ps[n][:], ones[:], W[n][1][:],
                                     start=False, stop=True)
                    dt_n = f32 if n[0] == "v" else qk_dt
                    t_sb = qkv_pool.tile([128, NF], dt_n, tag=f"s_{n}", name=f"s_{n}")
                    evac_prio = int(os.environ.get("BIDIR_EVAC_PRIO", "0"))
                    from contextlib import nullcontext
                    prio_ctx = (tc.high_priority(offset=evac_prio)
                                if evac_prio and n[0] != "v" else nullcontext())
                    with prio_ctx:
                        if n[0] == "q":
                            nc.scalar.mul(t_sb[:], ps[n][:], 1.0 / SCALE)
                        else:
                            nc.scalar.copy(t_sb[:], ps[n][:])
                    sb[n] = t_sb
                q, k, v = sb[qn], sb[kn], sb[vn]
                prod = big_pool.tile([128, H, H, DH], qk_dt, tag="bigbuf", name="bigbuf")
                q_b = (q[:].rearrange("p (h d) -> p h d", h=H)
                       .unsqueeze(2).broadcast_to([128, H, H, DH]))
                k_b = (k[:].rearrange("p (e d) -> p e d", e=H)
                       .unsqueeze(1).broadcast_to([128, H, H, DH]))
                sc = int(os.environ.get("BIDIR_SC_SPLIT", "8"))
                if sc >= 8:
                    nc.vector.tensor_mul(prod[:], q_b, k_b)
                else:
                    nc.vector.tensor_mul(prod[:, 0:sc], q_b[:, 0:sc], k_b[:, 0:sc])
                    nc.gpsimd.tensor_mul(prod[:, sc:8], q_b[:, sc:8], k_b[:, sc:8])
                s_t = small_pool.tile([128, H * H], f32, tag="s_t", name="s_t")
                if PROD_BF16:
                    # bf16 tree-adds run at 2x; final 8-wide reduce to fp32
                    tr1 = prod_pool.tile([128, H, H, 32], bf16, tag="trA", name="tr1")
                    tr1_eng = nc.gpsimd if int(os.environ.get("BIDIR_TR1_GP", "0")) else nc.vector
                    tr1_eng.tensor_add(tr1[:], prod[:, :, :, 0:32],
                                       prod[:, :, :, 32:64])
                    tr2 = prod_pool.tile([128, H, H, 16], bf16, tag="trB", name="tr2")
                    nc.vector.tensor_add(tr2[:], tr1[:, :, :, 0:16],
                                         tr1[:, :, :, 16:32])
                    tr3 = prod_pool.tile([128, H, H, 8], bf16, tag="trC", name="tr3")
                    nc.vector.tensor_add(tr3[:], tr2[:, :, :, 0:8],
                                         tr2[:, :, :, 8:16])
                    if int(os.environ.get("BIDIR_TREE_TAIL", "0")):
                        tr4 = prod_pool.tile([128, H, H, 4], bf16, tag="trD", name="tr4")
                        nc.vector.tensor_add(tr4[:], tr3[:, :, :, 0:4],
                                             tr3[:, :, :, 4:8])
                        tr5 = prod_pool.tile([128, H, H, 2], bf16, tag="trE", name="tr5")
                        nc.vector.tensor_add(tr5[:], tr4[:, :, :, 0:2],
                                             tr4[:, :, :, 2:4])
                        nc.vector.tensor_add(
                            s_t[:].rearrange("p (h e) -> p h e", h=H).unsqueeze(3),
                            tr5[:, :, :, 0:1], tr5[:, :, :, 1:2])
                    else:
                        nc.vector.tensor_reduce(s_t[:], tr3[:], axis=X, op=ADD)
                else:
                    nc.vector.tensor_reduce(s_t[:], prod[:], axis=X, op=ADD)
                e_t = small_pool.tile([128, H * H], f32, tag="e_t", name="e_t")
                nc.scalar.activation(e_t[:], s_t[:], EXP)
                if int(os.environ.get("BIDIR_PRE_NORM", "0")):
                    # normalize the attention weights at FD=64 (cheap) before
                    # the A@V product; the 2-slot pipeline hides the extra
                    # chain latency ahead of the GP launch
                    z_t = small_pool.tile([128, H], f32, tag="z_t", name="z_t")
                    nc.vector.tensor_reduce(
                        z_t[:], e_t[:].rearrange("p (h e) -> p h e", h=H),
                        axis=X, op=ADD)
                    r_t = small_pool.tile([128, H], f32, tag="r_t", name="r_t")
                    nc.vector.reciprocal(r_t[:], z_t[:])
                    a_t = small_pool.tile([128, H * H], f32, tag="a_t", name="a_t")
                    nc.vector.tensor_mul(
                        a_t[:].rearrange("p (h e) -> p h e", h=H),
                        e_t[:].rearrange("p (h e) -> p h e", h=H),
                        r_t[:].unsqueeze(2).broadcast_to([128, H, H]))
                    aw = a_t
                else:
                    aw = e_t
                p2 = big_pool.tile([128, H, DH, H], p2_dt, tag="bigbuf", name="bigbuf2")
                a_b = (aw[:].rearrange("p (h e) -> p h e", h=H)
                       .unsqueeze(2).broadcast_to([128, H, DH, H]))
                v_b = (v[:].rearrange("p (e d) -> p d e", e=H)
                       .unsqueeze(1).broadcast_to([128, H, DH, H]))
                if AV_ON_GPSIMD:
                    if int(os.environ.get("BIDIR_AV_HALVES", "0")):
                        # two GP half-ops: av-tree of half 0 can overlap half 1
                        nc.gpsimd.tensor_mul(p2[:, 0:4], a_b[:, 0:4], v_b[:, 0:4])
                        nc.gpsimd.tensor_mul(p2[:, 4:8], a_b[:, 4:8], v_b[:, 4:8])
                    else:
                        # split the A@V product GPSIMD/DVE to balance engines
                        hs = int(os.environ.get("BIDIR_AV_SPLIT", "6"))
                        if hs >= 8:
                            nc.gpsimd.tensor_mul(p2[:], a_b, v_b)
                        elif int(os.environ.get("BIDIR_AV_EXP", "0")):
                            # ACT pre-expands the DVE heads' weights to a
                            # contiguous bf16 tile so the DVE mul hits 2x
                            nh = 8 - hs
                            nc.gpsimd.tensor_mul(p2[:, 0:hs], a_b[:, 0:hs], v_b[:, 0:hs])
                            e_exp = prod_pool.tile([128, nh, H, DH], bf16,
                                                   tag="e_exp", name="e_exp")
                            nc.scalar.copy(
                                e_exp[:],
                                aw[:].rearrange("p (h e) -> p h e", h=H)[:, hs:8]
                                .unsqueeze(3).broadcast_to([128, nh, H, DH]))
                            p2d = prod_pool.tile([128, nh, H, DH], bf16,
                                                 tag="p2d", name="p2d")
                            v_c = (v[:].rearrange("p (e d) -> p e d", e=H)
                                   .unsqueeze(1).broadcast_to([128, nh, H, DH]))
                            nc.vector.tensor_mul(p2d[:], e_exp[:], v_c)
                            st_extra = p2d
                        else:
                            nc.gpsimd.tensor_mul(p2[:, 0:hs], a_b[:, 0:hs], v_b[:, 0:hs])
                            nc.vector.tensor_mul(p2[:, hs:8], a_b[:, hs:8], v_b[:, hs:8])
                else:
                    nc.vector.tensor_mul(p2[:], a_b, v_b)
                return {"p2": p2, "d": d, "msl": msl, "aoT": aoT[d],
                        "e_t": e_t, "sup": sup,
                        "p2d": locals().get("st_extra")}

        def stage2a(st):
            """softmax denominator + A@V reduce + normalize (post GP)."""
            if True:
                p2, e_t = st["p2"], st["e_t"]
                pre_norm = int(os.environ.get("BIDIR_PRE_NORM", "0"))
                if not pre_norm:
                    z_t = small_pool.tile([128, H], f32, tag="z_t", name="z_t")
                    nc.vector.tensor_reduce(
                        z_t[:], e_t[:].rearrange("p (h e) -> p h e", h=H),
                        axis=X, op=ADD)
                    r_t = small_pool.tile([128, H], f32, tag="r_t", name="r_t")
                    nc.vector.reciprocal(r_t[:], z_t[:])
                o_u = out_pool.tile([128, NF], f32, tag="o_u", name="o_u")
                p2d = st.get("p2d")
                if PROD_BF16 and p2d is not None:
                    hs = int(os.environ.get("BIDIR_AV_SPLIT", "6"))
                    nh = 8 - hs
                    av1 = prod_pool.tile([128, hs, DH, 4], bf16, tag="trA", name="av1")
                    nc.vector.tensor_add(av1[:], p2[:, 0:hs, :, 0:4], p2[:, 0:hs, :, 4:8])
                    av2 = prod_pool.tile([128, hs, DH, 2], bf16, tag="trB", name="av2")
                    nc.vector.tensor_add(av2[:], av1[:, :, :, 0:2], av1[:, :, :, 2:4])
                    o_u_v = o_u[:].rearrange("p (h d) -> p h d", h=H)
                    nc.vector.tensor_add(o_u_v[:, 0:hs].unsqueeze(3),
                                         av2[:, :, :, 0:1], av2[:, :, :, 1:2])
                    avd1 = prod_pool.tile([128, nh, 4, DH], bf16, tag="trD", name="avd1")
                    nc.vector.tensor_add(avd1[:], p2d[:, :, 0:4, :], p2d[:, :, 4:8, :])
                    avd2 = prod_pool.tile([128, nh, 2, DH], bf16, tag="trE", name="avd2")
                    nc.vector.tensor_add(avd2[:], avd1[:, :, 0:2, :], avd1[:, :, 2:4, :])
                    nc.vector.tensor_add(o_u_v[:, hs:8].unsqueeze(2),
                                         avd2[:, :, 0:1, :], avd2[:, :, 1:2, :])
                elif PROD_BF16 and int(os.environ.get("BIDIR_AV_HALVES", "0")):
                    o_u_v = o_u[:].rearrange("p (h d) -> p h d", h=H).unsqueeze(3)
                    for lo in (0, 4):
                        avh1 = prod_pool.tile([128, 4, DH, 4], bf16, tag="trA", name="avh1")
                        nc.vector.tensor_add(avh1[:], p2[:, lo:lo+4, :, 0:4],
                                             p2[:, lo:lo+4, :, 4:8])
                        avh2 = prod_pool.tile([128, 4, DH, 2], bf16, tag="trB", name="avh2")
                        nc.vector.tensor_add(avh2[:], avh1[:, :, :, 0:2],
                                             avh1[:, :, :, 2:4])
                        nc.vector.tensor_add(o_u_v[:, lo:lo+4],
                                             avh2[:, :, :, 0:1], avh2[:, :, :, 1:2])
                elif PROD_BF16:
                    av1 = prod_pool.tile([128, H, DH, 4], bf16, tag="trA", name="av1")
                    av1_eng = nc.gpsimd if int(os.environ.get("BIDIR_AV1_GP", "0")) else nc.vector
                    av1_eng.tensor_add(av1[:], p2[:, :, :, 0:4], p2[:, :, :, 4:8])
                    av2 = prod_pool.tile([128, H, DH, 2], bf16, tag="trB", name="av2")
                    nc.vector.tensor_add(av2[:], av1[:, :, :, 0:2], av1[:, :, :, 2:4])
                    nc.vector.tensor_add(
                        o_u[:].rearrange("p (h d) -> p h d", h=H).unsqueeze(3),
                        av2[:, :, :, 0:1], av2[:, :, :, 1:2])
                else:
                    nc.vector.tensor_reduce(o_u[:], p2[:], axis=X, op=ADD)
                if pre_norm:
                    st["o_t"] = o_u
                else:
                    o_t = out_pool.tile([128, NF], f32, tag="o_t", name="o_t")
                    norm_eng = nc.gpsimd if int(os.environ.get("BIDIR_NORM_GP", "0")) else nc.vector
                    norm_eng.tensor_mul(
                        o_t[:].rearrange("p (h d) -> p h d", h=H),
                        o_u[:].rearrange("p (h d) -> p h d", h=H),
                        r_t[:].unsqueeze(2).broadcast_to([128, H, DH]))
                    st["o_t"] = o_t

        def stage2b(st):
            """transpose -> aoT staging; out-proj once a super-tile completes."""
            o_t, msl = st["o_t"], st["msl"]
            tp = pt.tile([128, NF], f32, tag="tp", name="tp")
            for c in range(4):
                nc.tensor.transpose(tp[:, 128 * c:128 * (c + 1)],
                                    o_t[:, 128 * c:128 * (c + 1)],
                                    ident[:])
            nc.scalar.copy(st["aoT"][:, :, msl],
                           tp[:].rearrange("p (c m) -> p c m", c=4))
            sup = st["sup"]
            sup["done"] += 1
            if sup["done"] == 2 * N_MT:
                out_projections(sup)

        def out_projections(sup):
            s, aoT = sup["s"], sup["aoT"]
            ssl = slice(ST * s, ST * (s + 1))
            for d, n in ((1, "o1"), (2, "o2")):
                od = dr["o1T"] if d == 1 else dr["o2T"]
                for o in range(4):
                    op_ps = po.tile([128, NF], f32, tag="op", name="op")
                    for c in range(4):
                        nc.tensor.matmul(op_ps[:],
                                         WO[n][c][:, 128 * o:128 * (o + 1)],
                                         aoT[d][:, c, :],
                                         start=(c == 0), stop=(c == 3))
                    ob = out_pool.tile([128, NF], f32, tag="ob", name="ob")
                    nc.scalar.activation(ob[:], op_ps[:], IDENT_FN,
                                         bias=bo_sb[n][:, o:o + 1], scale=1.0)
                    nc.sync.dma_start(od.ap()[128 * o:128 * (o + 1), ssl], ob[:])

        from collections import deque
        _lag = int(os.environ.get("BIDIR_LAG", "2"))
        pipe = deque()
        for s in range(N_ST):
            sup = load_supertile(s)
            for t in range(N_MT):
                for d, (qn, kn, vn) in ((1, ("q1", "k1", "v1")),
                                        (2, ("q2", "k2", "v2"))):
                    st = stage1(sup, t, d, qn, kn, vn)
                    pipe.append(st)
                    if len(pipe) >= _lag + 1:
                        stage2a(pipe[-(_lag + 1)])
                    if len(pipe) >= _lag + 2:
                        stage2b(pipe.popleft())
        for st in list(pipe)[-_lag:]:
            stage2a(st)
        while pipe:
            stage2b(pipe.popleft())


def _get_module():
    if "nc" in _CACHE:
        return _CACHE["nc"]
    nc = bacc.Bacc("TRN2", target_bir_lowering=False, debug=False,
                   enable_asserts=True, num_devices=NCORES)
    dr = {}
    dr["xT"] = nc.dram_tensor("xT", [NF, BC], f32r, kind="ExternalInput")
    dr["yT"] = nc.dram_tensor("yT", [NF, BC], f32r, kind="ExternalInput")
    for n in WNAMES:
        dr[f"w_{n}"] = nc.dram_tensor(f"w_{n}", [NF + 1, NF], f32r,
                                      kind="ExternalInput")
    dr["wo1T"] = nc.dram_tensor("wo1T", [NF, NF], f32r, kind="ExternalInput")
    dr["wo2T"] = nc.dram_tensor("wo2T", [NF, NF], f32r, kind="ExternalInput")
    dr["bo1c"] = nc.dram_tensor("bo1c", [128, 4], f32, kind="ExternalInput")
    dr["bo2c"] = nc.dram_tensor("bo2c", [128, 4], f32, kind="ExternalInput")
    dr["ones_row"] = nc.dram_tensor("ones_row", [1, MT], f32r,
                                    kind="ExternalInput")
    dr["ident"] = nc.dram_tensor("ident", [128, 128], f32, kind="ExternalInput")
    dr["o1T"] = nc.dram_tensor("o1T", [NF, BC], f32, kind="ExternalOutput")
    dr["o2T"] = nc.dram_tensor("o2T", [NF, BC], f32, kind="ExternalOutput")

    with tile.TileContext(nc) as tc:
        _emit(nc, tc, dr)
    nc.compile()
    _CACHE["nc"] = nc
    return nc


def _prepare_in_maps(inputs):
    prot = np.asarray(inputs["protein_features"], dtype=np.float32)
    lig = np.asarray(inputs["ligand_features"], dtype=np.float32)

    shared = {}
    for n in WNAMES:
        wt = np.asarray(inputs[f"W{n}"], dtype=np.float32).T
        bt = np.asarray(inputs[f"b{n}"], dtype=np.float32)[None, :]
        shared[f"w_{n}"] = np.ascontiguousarray(np.concatenate([wt, bt], 0))
    shared["wo1T"] = np.ascontiguousarray(
        np.asarray(inputs["Wo1"], dtype=np.float32).T)
    shared["wo2T"] = np.ascontiguousarray(
        np.asarray(inputs["Wo2"], dtype=np.float32).T)
    shared["bo1c"] = np.ascontiguousarray(
        np.asarray(inputs["bo1"], dtype=np.float32).reshape(4, 128).T)
    shared["bo2c"] = np.ascontiguousarray(
        np.asarray(inputs["bo2"], dtype=np.float32).reshape(4, 128).T)
    shared["ones_row"] = np.ones((1, MT), dtype=np.float32)
    shared["ident"] = np.eye(128, dtype=np.float32)

    in_maps = []
    for c in range(NCORES):
        sl = slice(c * BC, (c + 1) * BC)
        m = dict(shared)
        m["xT"] = np.ascontiguousarray(prot[sl].T)
        m["yT"] = np.ascontiguousarray(lig[sl].T)
        in_maps.append(m)
    return in_maps


def _run(inputs, trace=False, tmpdir=None):
    nc = _get_module()
    in_maps = _prepare_in_maps(inputs)
    res = bass_utils.run_bass_kernel_spmd(
        nc, in_maps, core_ids=list(range(NCORES)), trace=trace, tmpdir=tmpdir)

    p_out = np.empty((B, NF), dtype=np.float32)
    l_out = np.empty((B, NF), dtype=np.float32)
    for c in range(NCORES):
        sl = slice(c * BC, (c + 1) * BC)
        p_out[sl] = res.results[c]["o1T"].T
        l_out[sl] = res.results[c]["o2T"].T
    return (p_out, l_out), res


def kernel(**inputs):
    out, _ = _run(inputs, trace=bool(int(os.environ.get("BIDIR_TRACE", "0"))))
    return out

